# revision 1
# baseline (speedup 1.0000x reference)
"""Trainium2 Bass kernel for LogicGatedSNN.

Computes: spikes = (spike_input @ ternarize(synapse_states).T >= 1.0)
  where ternarize(s) = +1 if s > 1, -1 if s < -1, else 0.

Strategy:
  - Data-parallel over the batch dim across 8 NeuronCores (1024 rows/core),
    weights replicated. No collectives.
  - Per core:
    * X [1024, 4096] f32 is transposed on the TensorE (128x128 tiles via
      identity matmul into PSUM) during the pipeline-fill window, then split
      into bf16 hi/lo planes (x = hi + lo exactly at bf16 pair precision,
      residual ~2^-18 * x) by ACT casts + DVE subtract, landing directly in
      k-major resident SBUF tiles.
    * W is ternarized to bf16 {-1,0,+1} (exact) with two DVE compares + add,
      staged to DRAM scratch per 512-row slab, and reloaded k-major with the
      xbar transpose-DMA (2-byte dtype), pipelined one slab ahead of compute.
    * Matmul: psum[b_tile 128, j 512] accumulated over k (32 k-tiles x
      hi/lo passes); stationary = X^T tiles [128k, 128b] (FWL bf16 weight
      loads), moving = W'^T [128k, 512j], N=512 per PSUM bank.
    * Spike threshold (>= 1.0) on DVE straight out of PSUM; output stored in
      natural [b, j] layout (f32 0/1), no host-side reshuffling.
  - bf16 hi+lo double-pass keeps the matmul exact to ~2^-18 relative per
    term, so the only output mismatches vs the fp32 reference are the
    ~1e-6-ambiguous threshold boundary cases (~20 of 33.5M outputs).
"""

import sys

if "/opt/trn_rl_repo" not in sys.path:
    sys.path.insert(0, "/opt/trn_rl_repo")

import numpy as np

N_CORES = 8
BATCH, IN_F, OUT_F = 8192, 4096, 4096
B_CORE = BATCH // N_CORES  # 1024

_BUILT = None


def build_bass(B, K, J, JS=512, KCH=1024, reps=1, ring_split=False, w_first=True,
               BH=512, xt_bufs=1, psum_bufs=2, out_ring=True, WBLK=None, TG=4):
    """Build the per-core Bass program for x:[B,K] f32, w:[J,K] f32 -> out:[B,J] f32.

    reps > 1 repeats the whole compute (idempotent) for benchmarking via
    wall-clock deltas between builds with different reps.
    """
    from concourse import bacc
    import concourse.mybir as mybir
    import concourse.tile as tile
    from concourse.bass import ts

    f32, bf16 = mybir.dt.float32, mybir.dt.bfloat16
    alu = mybir.AluOpType
    P = 128
    JS = min(JS, J)
    KCH = min(KCH, K)
    BH = min(BH, B)           # batch rows per pass
    NBP = B // BH             # number of batch passes
    BT = BH // P              # 128-row tiles per pass
    KT = K // P               # k tiles (partition-dim groups)
    NSLAB = J // JS           # output-feature slabs
    NKC = K // KCH            # staging chunks along k
    assert B % BH == 0 and BH % P == 0 and K % P == 0 and J % JS == 0

    nc = bacc.Bacc("TRN2", target_bir_lowering=False, debug=False)
    x = nc.dram_tensor("x", [B, K], f32, kind="ExternalInput")
    w = nc.dram_tensor("w", [J, K], f32, kind="ExternalInput")
    out = nc.dram_tensor("out", [B, J], f32, kind="ExternalOutput")

    with tile.TileContext(nc) as tc:
        with (
            tc.tile_pool(name="dram", bufs=1, space="DRAM") as dpool,
            tc.tile_pool(name="xstage32", bufs=2) as xs32,
            tc.tile_pool(name="xstage16", bufs=2) as xs16,
            tc.tile_pool(name="wstage32", bufs=2) as ws32,
            tc.tile_pool(name="wstage16", bufs=2) as ws16,
            tc.tile_pool(name="xtres", bufs=xt_bufs) as xtres,
            tc.tile_pool(name="wtp", bufs=2) as wtp,
            tc.tile_pool(name="ostage", bufs=8) as op,
            tc.tile_pool(name="psum", bufs=1, space="PSUM") as pp,
        ):
            # DRAM scratch: ternarized W (natural layout). Separate tiles per
            # row-block keep RAW deps slab-granular for pipelining.
            if WBLK is None:
                WBLK = JS  # ternarize/consume dependency granularity
            WBLK = min(WBLK, JS)
            assert JS % WBLK == 0
            wt_blocks = [
                dpool.tile([WBLK, K], bf16, name=f"wt_nat_r{r}")
                for r in range(J // WBLK)
            ]

            import itertools

            from concourse.masks import make_identity

            ident = xtres.tile([P, P], f32, name="ident")
            make_identity(nc, ident[:])

            xdma = nc.scalar if ring_split else nc.sync
            odma = nc.scalar if out_ring else nc.sync

            def tern_slab_rows(j0, js):
                # ternarize W rows [j0, j0+js) into wt_blocks
                for jsub in range(js // P):
                    jj = j0 + jsub * P
                    for kc in range(NKC):
                        c0 = kc * KCH
                        win = ws32.tile([P, KCH], f32, name="win")
                        nc.sync.dma_start(
                            out=win[:], in_=w[jj : jj + P, c0 : c0 + KCH]
                        )
                        a = ws16.tile([P, KCH], bf16, name="wpos")
                        nc.vector.tensor_scalar(
                            out=a[:], in0=win[:], scalar1=1.0, scalar2=None,
                            op0=alu.is_gt,
                        )
                        b2 = ws16.tile([P, KCH], bf16, name="wneg")
                        nc.vector.tensor_scalar(
                            out=b2[:], in0=win[:], scalar1=-1.0, scalar2=-1.0,
                            op0=alu.is_ge, op1=alu.add,
                        )
                        t = ws16.tile([P, KCH], bf16, name="wtern")
                        nc.vector.tensor_add(out=t[:], in0=a[:], in1=b2[:])
                        nc.sync.dma_start(
                            out=wt_blocks[jj // WBLK][
                                jj % WBLK : jj % WBLK + P, c0 : c0 + KCH
                            ],
                            in_=t[:],
                        )

            def wt_t_load(wt, j0, js):
                # transpose-load W'^T rows [j0, j0+js) into wt[:, :, 0:js]
                assert j0 % WBLK == 0 and js % WBLK == 0
                for i in range(js // WBLK):
                    nc.sync.dma_start_transpose(
                        out=wt[:, :, i * WBLK : (i + 1) * WBLK],
                        in_=wt_blocks[j0 // WBLK + i][:],
                    )

            # TG: k-tiles per PSUM transpose group (TG=4 -> one 2KB bank)
            HEAD_SPLIT = False

            for rep, bp in itertools.product(range(reps), range(NBP)):
                if w_first and bp == 0 and rep == 0:
                    tern_slab_rows(0, JS)
                # ---- X prep: PE-transpose 128x128 f32 tiles into PSUM,
                # then split hi/lo (ACT casts + DVE subtract) straight into
                # the resident k-major column tiles. No DRAM round trip.
                # xtc[b][:, kt, 0:P] = hi, [:, kt, P:2P] = lo.
                xtc = [
                    xtres.tile([P, KT, 2 * P], bf16, name=f"xtc{bsub}")
                    for bsub in range(BT)
                ]
                for bsub in range(BT):
                    r0 = bp * BH + bsub * P
                    xin = xs32.tile([P, K], f32, name="xin")
                    xdma.dma_start(out=xin[:], in_=x[r0 : r0 + P, :])
                    for g in range(KT // TG):
                        tp = pp.tile(
                            [P, TG, P], f32, name="tps",
                            bufs=max(1, (8 - BT) * 4 // max(4, TG)),
                        )
                        for i in range(TG):
                            kt = g * TG + i
                            nc.tensor.transpose(
                                tp[:, i, :], xin[:, kt * P : (kt + 1) * P], ident[:]
                            )
                        hi_sl = xtc[bsub][:, g * TG : (g + 1) * TG, 0:P]
                        nc.scalar.copy(out=hi_sl, in_=tp[:])
                        h32 = xs32.tile([P, TG, P], f32, name="h32")
                        nc.scalar.copy(out=h32[:], in_=hi_sl)
                        nc.vector.tensor_sub(
                            out=xtc[bsub][:, g * TG : (g + 1) * TG, P : 2 * P],
                            in0=tp[:],
                            in1=h32[:],
                        )

                # First slab of the first batch pass is split small so the
                # first matmuls only wait on a 128-row ternarize chain.
                if bp == 0 and HEAD_SPLIT:
                    slabs = [(0, P), (P, JS - P)] + [
                        (s * JS, JS) for s in range(1, NSLAB)
                    ]
                else:
                    slabs = [(s * JS, JS) for s in range(NSLAB)]
                for s, (j0, js) in enumerate(slabs):
                    if bp == 0 and not (w_first and j0 == 0 and rep == 0):
                        tern_slab_rows(j0, js)

                    # ---- transpose-load W'^T slab: [128 kpart, KT, js] ----
                    wt = wtp.tile([P, KT, JS], bf16, name="wt")
                    wt_t_load(wt, j0, js)

                    # ---- matmuls (k outer, b inner) + threshold/store ----
                    psums = [
                        pp.tile([P, JS], f32, name=f"acc{b}", bufs=1)
                        for b in range(BT)
                    ]
                    for k in range(KT):
                        for b in range(BT):
                            nc.tensor.matmul(
                                psums[b][:, 0:js],
                                xtc[b][:, k, 0:P],
                                wt[:, k, 0:js],
                                start=(k == 0),
                                stop=False,
                            )
                            nc.tensor.matmul(
                                psums[b][:, 0:js],
                                xtc[b][:, k, P : 2 * P],
                                wt[:, k, 0:js],
                                start=False,
                                stop=(k == KT - 1),
                            )
                    for b in range(BT):
                        spk = op.tile([P, JS], f32, name="spk")
                        nc.vector.tensor_scalar(
                            out=spk[:, 0:js], in0=psums[b][:, 0:js], scalar1=1.0,
                            scalar2=None, op0=alu.is_ge,
                        )
                        r0 = bp * BH + b * P
                        odma.dma_start(
                            out=out[r0 : r0 + P, j0 : j0 + js], in_=spk[:, 0:js]
                        )

    nc.compile()
    return nc


def _get_built():
    global _BUILT
    if _BUILT is None:
        _BUILT = build_bass(B_CORE, IN_F, OUT_F)
    return _BUILT


def kernel(spike_input: np.ndarray, synapse_states: np.ndarray) -> np.ndarray:
    from concourse.bass_utils import run_bass_kernel_spmd

    nc = _get_built()
    xs = np.ascontiguousarray(spike_input, dtype=np.float32)
    ws = np.ascontiguousarray(synapse_states, dtype=np.float32)
    in_maps = [
        {"x": xs[c * B_CORE : (c + 1) * B_CORE], "w": ws} for c in range(N_CORES)
    ]
    res = run_bass_kernel_spmd(nc, in_maps, core_ids=list(range(N_CORES)))
    out = np.empty((BATCH, OUT_F), dtype=np.float32)
    for c in range(N_CORES):
        out[c * B_CORE : (c + 1) * B_CORE] = res.results[c]["out"]
    return out



# revision 11
# speedup vs baseline: 2.3227x; 2.3227x over previous
"""Trainium2 Bass kernel for LogicGatedSNN.

Computes: spikes = (spike_input @ ternarize(synapse_states).T >= 1.0)
  where ternarize(s) = +1 if s > 1, -1 if s < -1, else 0.

Strategy:
  - 4x2 grid over 8 NeuronCores: batch split 4 ways (2048 rows/core),
    out_features split 2 ways (2048 cols/core). No collectives; the full
    output is assembled host-side from disjoint blocks. The 2-way W split
    halves per-core HBM traffic for W / ternarized-W versus pure data
    parallelism, and halves the DVE ternarize work.
  - Per core, BH=1024 batch rows per pass (2 passes), JS=256 output slabs:
    * X [2048, 4096] f32 is transposed on the TensorE (128x128 tiles via
      identity matmul into PSUM), then copied into k-major resident SBUF
      tiles (full f32 — no hi/lo split). 8 resident X^T tiles per pass.
    * W is ternarized to bf16 {-1,0,+1} (exact) with two DVE compares + add,
      staged to DRAM scratch per 512-row slab, reloaded k-major with the
      xbar transpose-DMA (2-byte dtype), then upcast bf16->f32 on the
      Scalar engine per k-tile just ahead of the matmuls. Ternarized W is
      re-read once per batch pass (2x total).
    * Matmul: single pass in float32r (PE "relaxed fp32": 1 col/cycle at
      moving free-dim >= 256, ~tf32-class precision — measured rms error
      3e-3 on K=4096 ternary accumulation, well inside the 2e-2 gate).
      Stationary = X^T tiles [128k, 128b], moving = W'^T [128k, 256j]
      (f32 tiles, APs bitcast to float32r), accumulated over 32 k-tiles.
      PSUM: two 256-col accumulators share each 2KB bank as ONE
      accumulation group (start on the even half's first matmul, stop on
      the odd half's last) so 8 batch tiles fit in 4 banks, leaving 4 for
      the X-transpose staging.
    * Spike threshold (>= 1.0) on DVE straight out of PSUM (one op per
      bank pair), emitted as bf16 0/1 (exact) to halve the output DMA;
      host casts back to f32.
  - vs the bf16 hi/lo double-pass baseline: half the PE matmul work.
"""

import sys

if "/opt/trn_rl_repo" not in sys.path:
    sys.path.insert(0, "/opt/trn_rl_repo")

import numpy as np

N_CORES = 8
BATCH, IN_F, OUT_F = 8192, 4096, 4096
GRID_B, GRID_J = 4, 2
B_CORE = BATCH // GRID_B  # 2048
J_CORE = OUT_F // GRID_J  # 2048

_BUILT = None


def build_bass(B, K, J, JS=256, KCH=1024, XCH=2048, reps=1, ring_split=False,
               w_first=True, BH=1024, xt_bufs=1, out_bf16=True, WBLK=256, TG=4,
               WFR=4):
    """Build the per-core Bass program for x:[B,K] f32, w:[J,K] f32 -> out:[B,J].

    reps > 1 repeats the whole compute (idempotent) for benchmarking via
    wall-clock deltas between builds with different reps.
    """
    from concourse import bacc
    import concourse.mybir as mybir
    import concourse.tile as tile

    f32, f32r, bf16 = mybir.dt.float32, mybir.dt.float32r, mybir.dt.bfloat16
    alu = mybir.AluOpType
    P = 128
    JS = min(JS, J)
    KCH = min(KCH, K)
    XCH = min(XCH, K)
    BH = min(BH, B)           # batch rows per pass
    NBP = B // BH             # number of batch passes
    BT = BH // P              # 128-row tiles per pass
    KT = K // P               # k tiles (partition-dim groups)
    NSLAB = J // JS           # output-feature slabs
    NKC = K // KCH            # W staging chunks along k
    NXC = K // XCH            # X staging chunks along k
    odt = bf16 if out_bf16 else f32
    assert B % BH == 0 and BH % P == 0 and K % P == 0 and J % JS == 0
    # Pair two accumulators per PSUM bank when a slab is a half-bank wide.
    pair = (JS * 4 * 2 <= 2048) and (BT % 2 == 0)

    nc = bacc.Bacc("TRN2", target_bir_lowering=False, debug=False)
    x = nc.dram_tensor("x", [B, K], f32, kind="ExternalInput")
    w = nc.dram_tensor("w", [J, K], f32, kind="ExternalInput")
    out = nc.dram_tensor("out", [B, J], odt, kind="ExternalOutput")

    with tile.TileContext(nc) as tc:
        with (
            tc.tile_pool(name="dram", bufs=1, space="DRAM") as dpool,
            tc.tile_pool(name="xstage32", bufs=2) as xs32,
            tc.tile_pool(name="wstage32", bufs=2) as ws32,
            tc.tile_pool(name="wstage16", bufs=2) as ws16,
            tc.tile_pool(name="xtres", bufs=xt_bufs) as xtres,
            tc.tile_pool(name="wtp", bufs=2) as wtp,
            tc.tile_pool(name="wfp", bufs=WFR) as wfp,
            tc.tile_pool(name="ostage", bufs=6) as op,
            tc.tile_pool(name="psum", bufs=1, space="PSUM") as pp,
        ):
            # DRAM scratch: ternarized W (natural layout). Separate tiles per
            # row-block keep RAW deps slab-granular for pipelining.
            WBLK = min(WBLK, J)
            wt_blocks = [
                dpool.tile([WBLK, K], bf16, name=f"wt_nat_r{r}")
                for r in range(J // WBLK)
            ]

            import itertools

            from concourse.masks import make_identity

            ident = xtres.tile([P, P], f32, name="ident")
            make_identity(nc, ident[:])

            xdma = nc.scalar if ring_split else nc.sync
            odma = nc.scalar

            def tern_rows(j0, js):
                # ternarize W rows [j0, j0+js) into wt_blocks
                for jsub in range(js // P):
                    jj = j0 + jsub * P
                    for kc in range(NKC):
                        c0 = kc * KCH
                        win = ws32.tile([P, KCH], f32, name="win")
                        nc.sync.dma_start(
                            out=win[:], in_=w[jj : jj + P, c0 : c0 + KCH]
                        )
                        a = ws16.tile([P, KCH], bf16, name="wpos")
                        nc.vector.tensor_scalar(
                            out=a[:], in0=win[:], scalar1=1.0, scalar2=None,
                            op0=alu.is_gt,
                        )
                        b2 = ws16.tile([P, KCH], bf16, name="wneg")
                        nc.vector.tensor_scalar(
                            out=b2[:], in0=win[:], scalar1=-1.0, scalar2=-1.0,
                            op0=alu.is_ge, op1=alu.add,
                        )
                        t = ws16.tile([P, KCH], bf16, name="wtern")
                        nc.vector.tensor_add(out=t[:], in0=a[:], in1=b2[:])
                        nc.sync.dma_start(
                            out=wt_blocks[jj // WBLK][
                                jj % WBLK : jj % WBLK + P, c0 : c0 + KCH
                            ],
                            in_=t[:],
                        )

            def wt_t_load(wt, j0, js):
                # transpose-load W'^T rows [j0, j0+js) into wt[:, :, 0:js]
                for i0 in range(0, js, WBLK):
                    blk = wt_blocks[(j0 + i0) // WBLK]
                    r = (j0 + i0) % WBLK
                    width = min(WBLK - r, js - i0)
                    nc.sync.dma_start_transpose(
                        out=wt[:, :, i0 : i0 + width],
                        in_=blk[r : r + width, :],
                    )

            # TG: k-tiles per PSUM transpose group (TG=4 -> one 2KB bank)
            acc_banks = (BT // 2) if pair else BT
            tp_bufs = max(1, min(4, 8 - acc_banks))

            # ternarize granularity: do a slab's worth of rows just ahead of
            # first use, except the leading chunk which is done up front.
            tern_done = [False] * (J // WBLK)

            def tern_upto(j_end):
                for r in range((j_end + WBLK - 1) // WBLK):
                    if not tern_done[r]:
                        tern_rows(r * WBLK, WBLK)
                        tern_done[r] = True

            for rep, bp in itertools.product(range(reps), range(NBP)):
                if bp == 0:
                    # re-ternarize every rep so reps-delta benchmarking charges
                    # the full W pipeline to each rep
                    for r in range(len(tern_done)):
                        tern_done[r] = False
                    if w_first:
                        tern_upto(min(WBLK, J))
                # ---- X prep: PE-transpose 128x128 f32 tiles into PSUM, then
                # ACT-copy into the resident k-major f32 tiles.
                xtc = [
                    xtres.tile([P, KT, P], f32r, name=f"xtc{bsub}")
                    for bsub in range(BT)
                ]
                for bsub in range(BT):
                    r0 = bp * BH + bsub * P
                    xck = []
                    for cx in range(NXC):
                        xin = xs32.tile([P, XCH], f32, name="xin")
                        xdma.dma_start(
                            out=xin[:], in_=x[r0 : r0 + P, cx * XCH : (cx + 1) * XCH]
                        )
                        xck.append(xin)
                    for g in range(KT // TG):
                        tp = pp.tile([P, TG, P], f32, name="tps", bufs=tp_bufs)
                        for i in range(TG):
                            kt = g * TG + i
                            xin = xck[(kt * P) // XCH]
                            o = (kt * P) % XCH
                            nc.tensor.transpose(
                                tp[:, i, :], xin[:, o : o + P], ident[:]
                            )
                        nc.scalar.copy(
                            out=xtc[bsub][:, g * TG : (g + 1) * TG, :], in_=tp[:]
                        )

                for s in range(NSLAB):
                    j0, js = s * JS, JS
                    if bp == 0:
                        tern_upto(min(j0 + 2 * JS if w_first else j0 + JS, J))

                    # ---- transpose-load W'^T slab: [128 kpart, KT, js] bf16,
                    # then upcast per k-tile to f32 on the Scalar engine.
                    wt = wtp.tile([P, KT, JS], bf16, name="wt")
                    wt_t_load(wt, j0, js)

                    # ---- matmuls (k outer, b inner) + threshold/store ----
                    if pair:
                        banks = [
                            pp.tile([P, 2 * JS], f32, name=f"accp{i}", bufs=1)
                            for i in range(BT // 2)
                        ]

                        def acc_ap(b):
                            return banks[b // 2][:, (b % 2) * JS : (b % 2) * JS + js]
                    else:
                        banks = [
                            pp.tile([P, max(JS, 512)], f32, name=f"acc{b}", bufs=1)
                            for b in range(BT)
                        ]

                        def acc_ap(b):
                            return banks[b][:, 0:js]
                    for k in range(KT):
                        wf = wfp.tile([P, JS], f32r, name="wf")
                        nc.scalar.copy(out=wf[:, 0:js], in_=wt[:, k, 0:js])
                        wfr = wf[:, 0:js]
                        for b in range(BT):
                            if pair:
                                st = k == 0 and (b % 2 == 0)
                                sp = k == KT - 1 and (b % 2 == 1)
                            else:
                                st, sp = k == 0, k == KT - 1
                            nc.tensor.matmul(
                                acc_ap(b),
                                xtc[b][:, k, :],
                                wfr,
                                start=st,
                                stop=sp,
                                skip_group_check=pair,
                            )
                    if pair:
                        for i in range(BT // 2):
                            spk = op.tile([P, 2 * JS], odt, name="spk")
                            nc.vector.tensor_scalar(
                                out=spk[:], in0=banks[i][:], scalar1=1.0,
                                scalar2=None, op0=alu.is_ge,
                            )
                            for h in range(2):
                                b = 2 * i + h
                                r0 = bp * BH + b * P
                                odma.dma_start(
                                    out=out[r0 : r0 + P, j0 : j0 + js],
                                    in_=spk[:, h * JS : (h + 1) * JS],
                                )
                    else:
                        for b in range(BT):
                            spk = op.tile([P, JS], odt, name="spk")
                            nc.vector.tensor_scalar(
                                out=spk[:, 0:js], in0=banks[b][:, 0:js],
                                scalar1=1.0, scalar2=None, op0=alu.is_ge,
                            )
                            r0 = bp * BH + b * P
                            odma.dma_start(
                                out=out[r0 : r0 + P, j0 : j0 + js],
                                in_=spk[:, 0:js],
                            )

    nc.compile()
    return nc


def _get_built():
    global _BUILT
    if _BUILT is None:
        _BUILT = build_bass(B_CORE, IN_F, J_CORE)
    return _BUILT


def make_in_maps(xs, ws):
    """Per-core input slices for the GRID_B x GRID_J layout."""
    maps = []
    for c in range(N_CORES):
        bi, ji = c // GRID_J, c % GRID_J
        maps.append(
            {
                "x": xs[bi * B_CORE : (bi + 1) * B_CORE],
                "w": ws[ji * J_CORE : (ji + 1) * J_CORE],
            }
        )
    return maps


def assemble(results):
    """Gather per-core output blocks into the full [BATCH, OUT_F] f32 array."""
    out = np.empty((BATCH, OUT_F), dtype=np.float32)
    for c in range(N_CORES):
        bi, ji = c // GRID_J, c % GRID_J
        out[
            bi * B_CORE : (bi + 1) * B_CORE, ji * J_CORE : (ji + 1) * J_CORE
        ] = np.asarray(results[c]["out"]).astype(np.float32)
    return out


def kernel(spike_input: np.ndarray, synapse_states: np.ndarray) -> np.ndarray:
    from concourse.bass_utils import run_bass_kernel_spmd

    nc = _get_built()
    xs = np.ascontiguousarray(spike_input, dtype=np.float32)
    ws = np.ascontiguousarray(synapse_states, dtype=np.float32)
    res = run_bass_kernel_spmd(
        nc, make_in_maps(xs, ws), core_ids=list(range(N_CORES))
    )
    return assemble(res.results)
